# revision 23
# baseline (speedup 1.0000x reference)
"""Trainium2 Bass kernel for BatchSquareDiagonal.

Computes out[b] = sum_n d[b, n] * x[b, n]^2 for x, d of shape [16384, 2048]
f32, returning [16384, 1] f32. Pure data parallel across 8 NeuronCores:
core c handles batch rows [c*2048, (c+1)*2048).

Blockless raw-bass pipeline (memory-bound; ~33.5 MB of input reads per
core; measured 93-111 us per run depending on HBM-stack contention, vs a
~94 us streaming roofline at the 358 GB/s per-core HBM limit):
  - INTERLEAVED row assignment: batch row b = 16*p + j lives on SBUF
    partition p, result column j. Loads are 128 x 16KB fully-contiguous
    descriptors per unit, and the final [128,16] -> [2048] result store is
    contiguous 64B per partition (vs. a 2048 x 4B scatter, whose HBM
    write-receipt tail alone cost ~9.5 us).
  - No nc.Block() and no start barrier: consumer-side semaphore clears +
    structural ordering margins replace them; first loads issue within
    ~7 us of NEFF start.
  - ScalarE (ACT) squares x, VectorE (DVE) scalar_tensor_tensor does
    sum(x^2 * d) per partition into res via the DVE accumulator,
    elementwise product discarded into a stride-0 dummy broadcast.
  - Every res column is produced by a single stt accumulator flush whose
    @complete increment gates the store DMA. Do NOT try to split the
    last tile and merge partials on-engine: every variant (DVE add or
    ACT add, drained, sem-gated, plain or accum output) intermittently
    read a stale partial (rel err up to 4e-2 on low-contention runs).
"""

import os
import sys

import numpy as np

for _p in ("/opt/trn_rl_repo", os.path.expanduser("~/.axon_site/_ro/trn_rl_repo")):
    if os.path.isdir(_p) and _p not in sys.path:
        sys.path.insert(0, _p)

N_CORES = 8
B, N = 16384, 2048
B_LOCAL = B // N_CORES  # 2048 rows per core
P = 128                 # SBUF partitions
J = B_LOCAL // P        # 16 result columns per partition (row b = 16p + j)
G = 2                   # tiles per full-size unit

_NC_CACHE = {}


def _build_nc_v3():
    """Blockless raw-bass pipeline, interleaved row layout.

    No nc.Block(): walrus appends its fixed clear-the-sem-file epilogue
    (~51 EventSemaphore writes per engine, ~7 us if serialized) after each
    engine's LAST user instruction. Without a block-end barrier, the idle
    engines (PE/PL) and early-finishing ones (ACT/DVE) run their clears
    DURING the stream; only Sync's 49 clears trail the store wait.

    (The epilogue turns out to begin with an all-engine gather butterfly,
    so the clears cannot actually overlap the stream — removing the Block
    still saves its end drain+barrier handshake. Semaphores stay
    auto-numbered: pinning them high, e.g. 248+, hangs the device.)
    """
    import concourse.bass as bass
    from concourse import bacc, mybir

    f32 = mybir.dt.float32
    nc = bacc.Bacc("TRN2", target_bir_lowering=False, debug=False)
    x = nc.declare_dram_parameter("vector", [B_LOCAL, N], f32, isOutput=False)
    d = nc.declare_dram_parameter("diag_values", [B_LOCAL, N], f32, isOutput=False)
    out = nc.declare_dram_parameter("out", [B_LOCAL, 1], f32, isOutput=True)

    # row b = 16p + j  ->  xw[p, j*N + n]; per-partition bytes contiguous
    xw = x.ap().rearrange("(p j) n -> p (j n)", j=J)
    dw = d.ap().rearrange("(p j) n -> p (j n)", j=J)
    outv = out.ap().rearrange("(p j) o -> p (j o)", j=J)  # [128, 16], contiguous

    NBX, NBD, NBS = 4, 4, 3  # slot counts: x, d, sq
    W = G * N                # 4096 cols per full unit
    x_slots = [nc.alloc_sbuf_tensor(f"xs{i}", [P, W], f32) for i in range(NBX)]
    d_slots = [nc.alloc_sbuf_tensor(f"ds{i}", [P, W], f32) for i in range(NBD)]
    s_slots = [nc.alloc_sbuf_tensor(f"ss{i}", [P, W], f32) for i in range(NBS)]
    res = nc.alloc_sbuf_tensor("k_res", [P, J], f32)
    dummy = nc.alloc_sbuf_tensor("k_dummy", [P, 1], f32)

    # units: u=0..6 cover tiles (2u, 2u+1); u=7 -> tile 14; u=8 -> tile 15.
    # x15 loads/squares in halves so ACT never gates the final stt; d15
    # loads last, and tile 15's single full-width stt (+store) is the only
    # work trailing the last input byte (~4.5 us).
    # dve count after unit u's stts, u=0..6; then stt14 -> 15, stt15 -> 16
    cum_stt = [2, 4, 6, 8, 10, 12, 14]
    DVE_DONE = 16
    # d of unit v fully read after (for d-slot reuse), v=0..4
    d_read_done = [2, 4, 6, 8, 10]
    H = N // 2

    def x_ap(u):
        t = x_slots[u % NBX].ap()
        return t if u < 7 else t[:, :N]

    def d_ap(u):
        t = d_slots[u % NBD].ap()
        return t if u < 7 else t[:, :N]

    def s_ap(u):
        t = s_slots[u % NBS].ap()
        return t if u < 7 else t[:, :N]

    def xsrc(u):
        if u < 7:
            return xw[:, u * W : (u + 1) * W]
        return xw[:, (7 + u) * N : (8 + u) * N]  # u=7 -> tile14, u=8 -> tile15

    def dsrc(u):
        if u < 7:
            return dw[:, u * W : (u + 1) * W]
        return dw[:, (7 + u) * N : (8 + u) * N]

    # One completion semaphore PER DMA. A shared counting sem with
    # wait >= 16*m is WRONG: each DMA incs +16 (one per SDMA engine), but
    # engines drain their per-engine FIFOs independently, so fast engines
    # ahead on DMA m+1 can push the total past 16*m while slow engines are
    # still delivering DMA m -- consumers then read engine-owned partition
    # groups of stale data (observed: intermittent rel err up to 7e-2 on
    # the tail tiles of low-contention runs). sem >= 16 on a single-DMA
    # sem is exact.
    xs_sem = [nc.alloc_semaphore(f"x{u}") for u in range(7)]
    ds_sem = [nc.alloc_semaphore(f"d{u}") for u in range(7)]
    x14_sem = nc.alloc_semaphore("x14")
    d14_sem = nc.alloc_semaphore("d14")
    x15a_sem = nc.alloc_semaphore("x15a")
    x15b_sem = nc.alloc_semaphore("x15b")
    d15_sem = nc.alloc_semaphore("d15")
    act_sem = nc.alloc_semaphore("act")
    dve_sem = nc.alloc_semaphore("dve")
    st_sem = nc.alloc_semaphore("st")

    sync, scalar, vector = nc.sync, nc.scalar, nc.vector
    rap = res.ap()

    # Consumer-side semaphore clears, no barrier needed: NRT does not zero
    # semaphores at NEFF start, but every engine's first wait on a sem is
    # ordered >=5 us after that sem's clear here (engine preambles end
    # barrier-synced within ~0.5 us of each other, and the first producer
    # increments land only after megabytes of DMA): each load sem is
    # cleared on its consuming engine, dve on scalar (sync's first dve
    # wait comes later still), act on vector, st on sync.
    for s in xs_sem + [x14_sem, x15a_sem, x15b_sem, d15_sem, dve_sem]:
        scalar.sem_clear(s)
    for s in ds_sem + [d14_sem, act_sem]:
        vector.sem_clear(s)
    sync.sem_clear(st_sem)

    # First unit's loads have no waits; issue immediately. Their sem
    # incs land only after ~2MB streams in, well after the clears.
    sync.dma_start(out=x_ap(0), in_=xsrc(0)).then_inc(xs_sem[0], 16)
    sync.dma_start(out=d_ap(0), in_=dsrc(0)).then_inc(ds_sem[0], 16)

    # --- sync: loads, result store ---
    for u in range(1, 7):
        if u >= NBX:
            sync.wait_ge(act_sem, u - NBX + 1)
        sync.dma_start(out=x_ap(u), in_=xsrc(u)).then_inc(xs_sem[u], 16)
        if u >= NBD:
            sync.wait_ge(dve_sem, d_read_done[u - NBD])
        sync.dma_start(out=d_ap(u), in_=dsrc(u)).then_inc(ds_sem[u], 16)
    # tail loads: x14(15), d14(16), x15 halves (17,18), d15(19)
    sync.wait_ge(act_sem, 4)  # x slot 3 free (unit 3's square done)
    sync.dma_start(out=x_ap(7), in_=xsrc(7)).then_inc(x14_sem, 16)
    sync.wait_ge(dve_sem, d_read_done[3])  # d slot 3 free
    sync.dma_start(out=d_ap(7), in_=dsrc(7)).then_inc(d14_sem, 16)
    sync.wait_ge(act_sem, 5)  # x slot 0 free
    sync.dma_start(out=x_ap(8)[:, :H], in_=xsrc(8)[:, :H]).then_inc(x15a_sem, 16)
    sync.dma_start(out=x_ap(8)[:, H:], in_=xsrc(8)[:, H:]).then_inc(x15b_sem, 16)
    sync.wait_ge(dve_sem, d_read_done[4])  # d slot 0 free
    sync.dma_start(out=d_ap(8), in_=dsrc(8)).then_inc(d15_sem, 16)
    sync.wait_ge(dve_sem, DVE_DONE)
    with nc.allow_non_contiguous_dma(reason="8KB result store"):
        sync.dma_start(out=outv, in_=res.ap()).then_inc(st_sem, 16)
    sync.wait_ge(st_sem, 16)

    # --- scalar: squares (units 0..6, tile14, then x15 in halves) ---
    for u in range(7):
        if u >= NBS:
            scalar.wait_ge(dve_sem, cum_stt[u - NBS])
        scalar.wait_ge(xs_sem[u], 16)
        scalar.square(s_ap(u), x_ap(u)).then_inc(act_sem, 1)
    scalar.wait_ge(dve_sem, cum_stt[4])  # s slot 1 free
    scalar.wait_ge(x14_sem, 16)
    scalar.square(s_ap(7), x_ap(7)).then_inc(act_sem, 1)  # act -> 8
    scalar.wait_ge(dve_sem, cum_stt[5])  # s slot 2 free
    scalar.wait_ge(x15a_sem, 16)
    scalar.square(s_ap(8)[:, :H], x_ap(8)[:, :H]).then_inc(act_sem, 1)  # -> 9
    scalar.wait_ge(x15b_sem, 16)
    scalar.square(s_ap(8)[:, H:], x_ap(8)[:, H:]).then_inc(act_sem, 1)  # -> 10

    # --- vector: fused mul+reduce ---
    def stt(sq_ap, dd_ap, accum_ap):
        return vector.scalar_tensor_tensor(
            out=dummy.ap().broadcast_to(sq_ap.shape),
            in0=sq_ap,
            scalar=1.0,
            in1=dd_ap,
            op0=mybir.AluOpType.mult,
            op1=mybir.AluOpType.mult,
            accum_out=accum_ap,
        )

    for u in range(7):
        vector.wait_ge(act_sem, u + 1)
        vector.wait_ge(ds_sem[u], 16)
        for g in range(G):
            j = G * u + g
            stt(
                s_ap(u)[:, bass.ts(g, N)],
                d_ap(u)[:, bass.ts(g, N)],
                rap[:, j : j + 1],
            ).then_inc(dve_sem, 1)
    # tile 14
    vector.wait_ge(act_sem, 8)
    vector.wait_ge(d14_sem, 16)
    stt(s_ap(7), d_ap(7), rap[:, 14:15]).then_inc(dve_sem, 1)  # dve -> 15
    # tile 15 as one full stt: every attempted split-and-merge of this
    # column (DVE add, ACT add, drained or sem-gated) intermittently read
    # a stale partial -- engine reads of fresh accumulator flushes are not
    # reliably ordered. One stt -> accum flush -> store DMA read is the
    # exact mechanism proven by columns 0..14.
    vector.wait_ge(act_sem, 10)  # both sq15 halves
    vector.wait_ge(d15_sem, 16)
    stt(s_ap(8), d_ap(8), rap[:, 15:16]).then_inc(dve_sem, 1)  # -> 16

    nc.finalize()
    return nc


def _build_nc_tile():
    """Tile-based fallback (previous session's kernel, proven correct)."""
    import concourse.bass as bass
    import concourse.tile as tile
    from concourse import bacc, mybir

    f32 = mybir.dt.float32
    nc = bacc.Bacc("TRN2", target_bir_lowering=False, debug=False)
    x = nc.declare_dram_parameter("vector", [B_LOCAL, N], f32, isOutput=False)
    d = nc.declare_dram_parameter("diag_values", [B_LOCAL, N], f32, isOutput=False)
    out = nc.declare_dram_parameter("out", [B_LOCAL, 1], f32, isOutput=True)

    N_TILES = B_LOCAL // P  # 16
    N_GROUPS = N_TILES // G

    xv = x.ap().rearrange("(t p) n -> t p n", p=P)
    dv = d.ap().rearrange("(t p) n -> t p n", p=P)
    outv = out.ap().rearrange("(j p) o -> p (j o)", p=P)

    with tile.TileContext(nc) as tc:
        with (
            tc.tile_pool(name="io", bufs=3) as io_pool,
            tc.tile_pool(name="acc", bufs=1) as acc_pool,
        ):
            res = acc_pool.tile([P, N_TILES], f32)
            dummy = acc_pool.tile([P, 1], f32)

            def fused_mul_sum(sq_ap, d_ap, accum_ap):
                nc.vector.scalar_tensor_tensor(
                    out=dummy.broadcast_to(sq_ap.shape),
                    in0=sq_ap,
                    scalar=1.0,
                    in1=d_ap,
                    op0=mybir.AluOpType.mult,
                    op1=mybir.AluOpType.mult,
                    accum_out=accum_ap,
                )

            x14 = io_pool.tile([P, N], f32, tag="x", bufs=4)
            d14 = io_pool.tile([P, N], f32, tag="d", bufs=4)
            s14 = io_pool.tile([P, N], f32, tag="sq", bufs=3)
            nc.sync.dma_start(out=x14, in_=xv[14])
            nc.sync.dma_start(out=d14, in_=dv[14])
            nc.scalar.square(s14, x14)
            fused_mul_sum(s14[:], d14[:], res[:, 14:15])

            x15 = io_pool.tile([P, N], f32, tag="x", bufs=4)
            d15 = io_pool.tile([P, N], f32, tag="d", bufs=4)
            s15 = io_pool.tile([P, N], f32, tag="sq", bufs=3)
            nc.sync.dma_start(out=x15, in_=xv[15])
            nc.sync.dma_start(out=d15, in_=dv[15])
            nc.scalar.square(s15, x15)
            fused_mul_sum(s15[:], d15[:], res[:, 15:16])

            for g in range(N_GROUPS - 1):
                xt = io_pool.tile([P, G * N], f32, tag="x", bufs=4)
                dt = io_pool.tile([P, G * N], f32, tag="d", bufs=4)
                sq = io_pool.tile([P, G * N], f32, tag="sq", bufs=3)
                xg = xv[G * g : G * g + G].transpose([1, 0, 2])
                dg = dv[G * g : G * g + G].transpose([1, 0, 2])
                nc.sync.dma_start(out=xt.rearrange("p (i n) -> p i n", i=G), in_=xg)
                nc.sync.dma_start(out=dt.rearrange("p (i n) -> p i n", i=G), in_=dg)
                nc.scalar.square(sq, xt)
                for i in range(G):
                    j = G * g + i
                    fused_mul_sum(
                        sq[:, bass.ts(i, N)], dt[:, bass.ts(i, N)], res[:, j : j + 1]
                    )
                if g == 2:
                    nc.gpsimd.dma_start(out=outv[:, 14:16], in_=res[:, 14:16])
                    nc.gpsimd.dma_start(out=outv[:, :6], in_=res[:, :6])

            nc.gpsimd.dma_start(out=outv[:, 6:12], in_=res[:, 6:12])
            nc.gpsimd.dma_start(out=outv[:, 12:14], in_=res[:, 12:14])

    nc.finalize()
    return nc


def _get_nc():
    if "nc" not in _NC_CACHE:
        builder = (
            _build_nc_tile if os.environ.get("TILE_KERNEL") == "1" else _build_nc_v3
        )
        _NC_CACHE["nc"] = builder()
    return _NC_CACHE["nc"]


def kernel(vector, diag_values):
    from concourse.bass_utils import run_bass_kernel_spmd

    vector = np.ascontiguousarray(np.asarray(vector, dtype=np.float32))
    diag_values = np.ascontiguousarray(np.asarray(diag_values, dtype=np.float32))
    assert vector.shape == (B, N) and diag_values.shape == (B, N)

    vs = vector.reshape(N_CORES, B_LOCAL, N)
    ds = diag_values.reshape(N_CORES, B_LOCAL, N)
    in_maps = [{"vector": vs[c], "diag_values": ds[c]} for c in range(N_CORES)]

    nc = _get_nc()
    res = run_bass_kernel_spmd(nc, in_maps, list(range(N_CORES)))
    return np.concatenate([res.results[c]["out"] for c in range(N_CORES)], axis=0)
